# revision 6
# baseline (speedup 1.0000x reference)
"""Trainium2 Bass kernel for causal self-attention (RoPE + per-head RMSNorm).

Reference computation (B=2, T=2048, C=1024, H=16, D=64):
    q = rope(rmsnorm(x @ Wq.T)); k = rope(rmsnorm(x @ Wk.T)); v = x @ Wv.T
    out = softmax(causal(q k^T / sqrt(D))) v @ Wo.T

Sharding over 8 NeuronCores: core c -> batch b = c//4, head-group g = c%4
(4 heads = 256 features per group).  Everything on-chip is computed in a
feature-major ("transposed") layout so no PE transposes are needed:
  - scores are computed as S^T[tk, tq] tiles, softmax runs over the
    partition axis using matmul-with-ones tricks (denominator comes from a
    ones column appended to V), and the final division is applied via a
    K=1 broadcast matmul.
  - attention output Y^T (feature-major) is exchanged with an AllToAll
    within each batch's 4-core group, giving each core the full 1024
    features for its 512-token slice; o_proj is computed on that slice.
Host side: shards/transposes inputs (bf16), assembles the fp32 output.
"""

import os
import sys

for _p in ("/opt/trn_rl_repo", "/root/.axon_site/_ro/trn_rl_repo"):
    if os.path.isdir(_p) and _p not in sys.path:
        sys.path.insert(0, _p)

import numpy as np
import ml_dtypes

import concourse.bass as bass
from concourse import bacc
import concourse.tile as tile
import concourse.mybir as mybir

BF16 = mybir.dt.bfloat16
F32 = mybir.dt.float32
AF = mybir.ActivationFunctionType

B, T, C, H, D = 2, 2048, 1024, 16, 64
N_CORES = 8
GH = 4  # heads per core
GF = GH * D  # features per core (256)
TB = 512  # token block (matmul N)
KT = C // 128  # 8 contraction k-tiles
EPS = float(np.finfo(np.float32).eps)
ROPE_BASE = 10000.0


def build_nc(t=T):
    ntb = t // TB  # tq blocks
    ntt = t // 128  # token 128-tiles
    tsl = t // 4  # per-core token slice for o_proj

    nc = bacc.Bacc("TRN2", target_bir_lowering=False, debug=False, num_devices=N_CORES)

    xt = nc.dram_tensor("xt", [C, t], BF16, kind="ExternalInput")
    wq = nc.dram_tensor("wq", [C, GF], BF16, kind="ExternalInput")
    wk = nc.dram_tensor("wk", [C, GF], BF16, kind="ExternalInput")
    wv = nc.dram_tensor("wv", [C, GF], BF16, kind="ExternalInput")
    wo = nc.dram_tensor("wo", [2 * C, C], BF16, kind="ExternalInput")
    cosf = nc.dram_tensor("cosf", [128, t], BF16, kind="ExternalInput")
    sinf = nc.dram_tensor("sinf", [128, t], BF16, kind="ExternalInput")
    pswap = nc.dram_tensor("pswap", [128, 128], BF16, kind="ExternalInput")
    blk2 = nc.dram_tensor("blk2", [128, 2], BF16, kind="ExternalInput")
    eqb = nc.dram_tensor("eqb", [2, 128], BF16, kind="ExternalInput")
    ekb = nc.dram_tensor("ekb", [2, 128], BF16, kind="ExternalInput")
    maskt = nc.dram_tensor("maskt", [128, 4 * TB], BF16, kind="ExternalInput")
    out = nc.dram_tensor("out", [C, tsl], F32, kind="ExternalOutput")

    with tile.TileContext(nc) as tc:
        with (
            nc.allow_low_precision(reason="bf16 compute by design"),
            tc.tile_pool(name="p_xt", bufs=KT) as p_xt,
            tc.tile_pool(name="p_w", bufs=KT) as p_w,
            tc.tile_pool(name="p_wo", bufs=2 * KT) as p_wo,
            tc.tile_pool(name="p_tab", bufs=1) as p_tab,
            tc.tile_pool(name="p_qk", bufs=2) as p_qk,
            tc.tile_pool(name="p_v", bufs=ntt) as p_v,
            tc.tile_pool(name="p_y", bufs=2) as p_y,
            tc.tile_pool(name="p_yg", bufs=2 * KT) as p_yg,
            tc.tile_pool(name="p_pt", bufs=4) as p_pt,
            tc.tile_pool(name="p_tmp", bufs=3) as p_tmp,
            tc.tile_pool(name="p_mm", bufs=6, space="PSUM") as p_mm,
            tc.tile_pool(name="p_oacc", bufs=2, space="PSUM") as p_oacc,
            tc.tile_pool(name="p_dram", bufs=2, space="DRAM") as p_dram,
        ):
            # ---- load inputs -------------------------------------------------
            xt_sb = []
            wq_sb = []
            wk_sb = []
            wv_sb = []
            for ct in range(KT):
                w_t = p_w.tile([128, GF], BF16, tag="wq")
                nc.sync.dma_start(w_t[:], wq[ct * 128 : (ct + 1) * 128, :])
                wq_sb.append(w_t)
                x_t = p_xt.tile([128, t], BF16, tag="xt")
                nc.sync.dma_start(x_t[:], xt[ct * 128 : (ct + 1) * 128, :])
                xt_sb.append(x_t)
            for ct in range(KT):
                w_t = p_w.tile([128, GF], BF16, tag="wk")
                nc.sync.dma_start(w_t[:], wk[ct * 128 : (ct + 1) * 128, :])
                wk_sb.append(w_t)
            for ct in range(KT):
                w_t = p_w.tile([128, GF], BF16, tag="wv")
                nc.sync.dma_start(w_t[:], wv[ct * 128 : (ct + 1) * 128, :])
                wv_sb.append(w_t)
            wo_sb = []
            for mt in range(2 * KT):
                w_t = p_wo.tile([128, C], BF16, tag="wo")
                nc.sync.dma_start(w_t[:], wo[mt * 128 : (mt + 1) * 128, :])
                wo_sb.append(w_t)

            cos_sb = p_tab.tile([128, t], BF16, tag="cos")
            nc.sync.dma_start(cos_sb[:], cosf[:])
            sin_sb = p_tab.tile([128, t], BF16, tag="sin")
            nc.sync.dma_start(sin_sb[:], sinf[:])
            pswap_sb = p_tab.tile([128, 128], BF16, tag="pswap")
            nc.sync.dma_start(pswap_sb[:], pswap[:])
            blk2_sb = p_tab.tile([128, 2], BF16, tag="blk2")
            nc.sync.dma_start(blk2_sb[:], blk2[:])
            eqb_sb = p_tab.tile([2, 128], BF16, tag="eqb")
            nc.sync.dma_start(eqb_sb[:], eqb[:])
            ekb_sb = p_tab.tile([2, 128], BF16, tag="ekb")
            nc.sync.dma_start(ekb_sb[:], ekb[:])
            mask_sb = p_tab.tile([128, 4 * TB], BF16, tag="mask")
            nc.sync.dma_start(mask_sb[:], maskt[:])
            ones64 = p_tab.tile([1, 64], BF16, tag="ones64")
            nc.vector.memset(ones64[:], 1.0)
            eps_sb = p_tab.tile([128, 1], F32, tag="eps")
            nc.vector.memset(eps_sb[:], EPS)

            # ---- q/k projections + rmsnorm + rope (feature-major) -----------
            def qk_tensor(w_sb, eb_sb):
                out_tiles = []
                for mt in range(2):  # 128-feature row groups (2 heads each)
                    qh_t = p_qk.tile([128, t], BF16, tag=f"qk{len(out_tiles)}")
                    for j in range(ntb):
                        jb = slice(j * TB, (j + 1) * TB)
                        pq = p_mm.tile([128, TB], F32, tag="mm")
                        for ct in range(KT):
                            nc.tensor.matmul(
                                pq[:],
                                w_sb[ct][:, mt * 128 : (mt + 1) * 128],
                                xt_sb[ct][:, jb],
                                start=(ct == 0),
                                stop=(ct == KT - 1),
                            )
                        # sum of squares per head (via blockdiag-ones matmul)
                        sq = p_tmp.tile([128, TB], BF16, tag="sq")
                        nc.scalar.activation(sq[:], pq[:], AF.Square)
                        pss = p_mm.tile([128, TB], F32, tag="mm")
                        nc.tensor.matmul(
                            pss[0:2, :], blk2_sb[:], sq[:], start=True, stop=True
                        )
                        sqm = p_tmp.tile([2, TB], F32, tag="sqm")
                        nc.scalar.activation(
                            sqm[:], pss[0:2, :], AF.Sqrt, scale=1.0 / D,
                            bias=eps_sb[0:2, :],
                        )
                        inv = p_tmp.tile([2, TB], BF16, tag="inv")
                        nc.vector.reciprocal(inv[:], sqm[:])
                        # broadcast inv over the 64 rows of each head (* norm w)
                        pinvb = p_mm.tile([128, TB], F32, tag="mm")
                        nc.tensor.matmul(
                            pinvb[:], eb_sb[:], inv[:], start=True, stop=True
                        )
                        invb = p_tmp.tile([128, TB], BF16, tag="invb")
                        nc.vector.tensor_copy(invb[:], pinvb[:])
                        qn = p_tmp.tile([128, TB], BF16, tag="qn")
                        nc.vector.tensor_mul(qn[:], pq[:], invb[:])
                        # rope: qh = qn*cos + swap(qn)*sin_signed
                        pqs = p_mm.tile([128, TB], F32, tag="mm")
                        nc.tensor.matmul(pqs[:], pswap_sb[:], qn[:], start=True, stop=True)
                        t1 = p_tmp.tile([128, TB], BF16, tag="t1")
                        nc.vector.tensor_mul(t1[:], qn[:], cos_sb[:, jb])
                        t2 = p_tmp.tile([128, TB], BF16, tag="t2")
                        nc.vector.tensor_mul(t2[:], pqs[:], sin_sb[:, jb])
                        nc.vector.tensor_add(qh_t[:, jb], t1[:], t2[:])
                    out_tiles.append(qh_t)
                return out_tiles

            qh_sb = qk_tensor(wq_sb, eqb_sb)
            kh_sb = qk_tensor(wk_sb, ekb_sb)

            # ---- v projection (token-major, [tk, 4x(64+ones)] layout) -------
            v_sb = []
            for tt in range(ntt):
                pv = p_mm.tile([128, TB], F32, tag="mm")
                for ct in range(KT):
                    nc.tensor.matmul(
                        pv[:, 0:GF],
                        xt_sb[ct][:, tt * 128 : (tt + 1) * 128],
                        wv_sb[ct][:],
                        start=(ct == 0),
                        stop=(ct == KT - 1),
                    )
                v_t = p_v.tile([128, GH * (D + 1)], BF16, tag="v")
                src = pv[:, 0:GF].rearrange("p (h d) -> p h d", h=GH)
                dst = v_t[:].rearrange("p (h d) -> p h d", h=GH, d=D + 1)
                nc.vector.tensor_copy(dst[:, :, 0:D], src)
                nc.vector.memset(dst[:, :, D : D + 1], 1.0)
                v_sb.append(v_t)

            # ---- attention + A2A exchange + o_proj --------------------------
            yg_sb = [None] * (2 * KT)
            bounce_in = []
            bounce_out = []
            for hp in range(2):  # head pairs (2 heads each)
                y_t = p_y.tile([128, t], BF16, tag="y")
                for j in range(ntb):
                    jb = slice(j * TB, (j + 1) * TB)
                    po = [
                        p_oacc.tile([D + 1, TB], F32, tag="oacc", name=f"po{i}")
                        for i in range(2)
                    ]
                    n_tt = 4 * (j + 1)
                    for tt in range(n_tt):
                        ps = [
                            p_mm.tile([128, TB], F32, tag="mm", name=f"ps{i}")
                            for i in range(2)
                        ]
                        for hl in range(2):  # head-in-pair
                            hofs = hl * 64
                            nc.tensor.matmul(
                                ps[hl][:],
                                kh_sb[hp][hofs : hofs + 64, tt * 128 : (tt + 1) * 128],
                                qh_sb[hp][hofs : hofs + 64, jb],
                                start=True,
                                stop=True,
                                tile_position=(hofs, 0),
                            )
                        for hl in range(2):
                            pt = p_pt.tile([128, TB], BF16, tag="pt")
                            nc.scalar.activation(
                                pt[:], ps[hl][:], AF.Exp, scale=1.0 / np.sqrt(D)
                            )
                            r = tt - 4 * j
                            if r >= 0:  # diagonal tile: apply causal mask
                                nc.vector.tensor_mul(
                                    pt[:], pt[:], mask_sb[:, r * TB : (r + 1) * TB]
                                )
                            h = 2 * hp + hl
                            nc.tensor.matmul(
                                po[hl][:],
                                v_sb[tt][:, h * (D + 1) : (h + 1) * (D + 1)],
                                pt[:],
                                start=(tt == 0),
                                stop=(tt == n_tt - 1),
                            )
                    # normalize: Y = O / den (den is row 64 of each po)
                    rec = [
                        p_tmp.tile([1, TB], BF16, tag=f"rec{i}", name=f"rec{i}")
                        for i in range(2)
                    ]
                    for hl in range(2):
                        nc.vector.reciprocal(rec[hl][:], po[hl][64:65, :])
                    pr = p_mm.tile([128, TB], F32, tag="mm")
                    nc.tensor.matmul(
                        pr[0:64, :], ones64[:], rec[0][:], start=True, stop=True,
                        tile_position=(0, 0),
                    )
                    nc.tensor.matmul(
                        pr[64:128, :], ones64[:], rec[1][:], start=True, stop=True,
                        tile_position=(0, 64),
                    )
                    r_sb = p_tmp.tile([128, TB], BF16, tag="rsb")
                    nc.vector.tensor_copy(r_sb[:], pr[:])
                    nc.vector.tensor_mul(y_t[0:64, jb], po[0][0:64, :], r_sb[0:64, :])
                    nc.vector.tensor_mul(
                        y_t[64:128, jb], po[1][0:64, :], r_sb[64:128, :]
                    )

                # exchange: 8-way AllToAll (4-core groups unsupported).
                # Shard s (dest rank s) carries our features(hp) for token
                # block s%4; rank c thus receives its token block c%4 from
                # every rank.  Rows from other-batch ranks are junk -- the
                # host zeroes the matching rows of wo so o_proj ignores them.
                bin_t = p_dram.tile([8 * 128, tsl], BF16, tag=f"bin{hp}")
                bout_t = p_dram.tile([8 * 128, tsl], BF16, tag=f"bout{hp}")
                bounce_in.append(bin_t)
                bounce_out.append(bout_t)
                for s in range(8):
                    i = s % 4
                    nc.gpsimd.dma_start(
                        bin_t[s * 128 : (s + 1) * 128, :],
                        y_t[:, i * tsl : (i + 1) * tsl],
                    )
                nc.gpsimd.collective_compute(
                    "AllToAll",
                    mybir.AluOpType.bypass,
                    ins=[bin_t.opt()],
                    outs=[bout_t.opt()],
                    replica_groups=[[0, 1, 2, 3, 4, 5, 6, 7]],
                )
                for i in range(8):
                    yg_t = p_yg.tile([128, tsl], BF16, tag="yg")
                    nc.sync.dma_start(yg_t[:], bout_t[i * 128 : (i + 1) * 128, :])
                    yg_sb[2 * i + hp] = yg_t

            # o_proj over our token slice: out^T[cout, tsl]
            for co in range(KT):
                pout = p_mm.tile([128, tsl], F32, tag="mm")
                for mt in range(2 * KT):
                    nc.tensor.matmul(
                        pout[:, 0:tsl],
                        wo_sb[mt][:, co * 128 : (co + 1) * 128],
                        yg_sb[mt][:],
                        start=(mt == 0),
                        stop=(mt == 2 * KT - 1),
                    )
                o_sb = p_tmp.tile([128, tsl], F32, tag="osb")
                nc.vector.tensor_copy(o_sb[:], pout[:, 0:tsl])
                nc.sync.dma_start(out[co * 128 : (co + 1) * 128, :], o_sb[:])

    nc.compile()
    return nc


# ---------------------------------------------------------------------------
# host side
# ---------------------------------------------------------------------------


def _rope_tables(t):
    inv_freq = 1.0 / (ROPE_BASE ** (np.arange(0, D, 2, dtype=np.float64) / D))  # [32]
    ang = np.arange(t, dtype=np.float64)[:, None] * inv_freq[None, :]  # [t, 32]
    cos = np.cos(ang).astype(np.float32)
    sin = np.sin(ang).astype(np.float32)
    cosf = np.empty((128, t), np.float32)
    sinf = np.empty((128, t), np.float32)
    for r in range(128):
        d = r % 64
        f = d if d < 32 else d - 32
        cosf[r] = cos[:, f]
        sinf[r] = -sin[:, f] if d < 32 else sin[:, f]
    return cosf, sinf


def _consts(t):
    cosf, sinf = _rope_tables(t)
    pswap = np.zeros((128, 128), np.float32)
    for j in range(128):
        d = j % 64
        i = (j - 32) if d >= 32 else (j + 32)
        pswap[i, j] = 1.0
    blk2 = np.zeros((128, 2), np.float32)
    blk2[0:64, 0] = 1.0
    blk2[64:128, 1] = 1.0
    maskt = np.zeros((128, 4 * TB), np.float32)
    for r in range(4):
        for p in range(128):
            lo = 128 * r + p
            if lo < TB:
                maskt[p, r * TB + lo : (r + 1) * TB] = 1.0
    return cosf, sinf, pswap, blk2, maskt


def _eb(w):
    e = np.zeros((2, 128), np.float32)
    e[0, 0:64] = w[0:64]
    e[1, 64:128] = w[0:64] if len(w) == 64 else w[64:128]
    return e


def _bf(x):
    return np.ascontiguousarray(x).astype(ml_dtypes.bfloat16)


def make_in_maps(x, Wq, Wk, Wv, Wo, qn_w, kn_w, t=T):
    cosf, sinf, pswap, blk2, maskt = _consts(t)
    eq = _eb(qn_w)
    ek = _eb(kn_w)
    common = {
        "cosf": _bf(cosf),
        "sinf": _bf(sinf),
        "pswap": _bf(pswap),
        "blk2": _bf(blk2),
        "eqb": _bf(eq),
        "ekb": _bf(ek),
        "maskt": _bf(maskt),
    }
    in_maps = []
    for c in range(N_CORES):
        b, g = c // 4, c % 4
        fs = slice(GF * g, GF * (g + 1))
        wot = Wo.T  # [c_in, c_out]
        wo_core = np.zeros((2 * C, C), np.float32)
        for i in range(8):
            if i // 4 == b:
                gi = i % 4
                for hp in range(2):
                    u = 256 * i + 128 * hp
                    f0 = GF * gi + 128 * hp
                    wo_core[u : u + 128, :] = wot[f0 : f0 + 128, :]
        in_maps.append(
            dict(
                common,
                xt=_bf(x[b, :t, :].T),
                wq=_bf(Wq[fs, :].T),
                wk=_bf(Wk[fs, :].T),
                wv=_bf(Wv[fs, :].T),
                wo=_bf(wo_core),
            )
        )
    return in_maps


def assemble(results, t=T):
    tsl = t // 4
    out = np.empty((B, t, C), np.float32)
    for c in range(N_CORES):
        b, g = c // 4, c % 4
        out[b, g * tsl : (g + 1) * tsl, :] = results[c]["out"].T
    return out


# -- cached PJRT runner (compile once, reuse across kernel() calls) ---------

_RUNNER = {}


def _get_runner(t=T):
    if t in _RUNNER:
        return _RUNNER[t]
    import jax
    from jax.sharding import Mesh, PartitionSpec
    from jax.experimental.shard_map import shard_map
    from concourse import bass2jax

    nc = build_nc(t)
    bass2jax.install_neuronx_cc_hook()

    partition_name = nc.partition_id_tensor.name if nc.partition_id_tensor else None
    in_names = []
    out_names = []
    out_avals = []
    zero_outs = []
    for alloc in nc.m.functions[0].allocations:
        if not isinstance(alloc, mybir.MemoryLocationSet):
            continue
        name = alloc.memorylocations[0].name
        if alloc.kind == "ExternalInput":
            if name == partition_name:
                continue
            in_names.append(name)
        elif alloc.kind == "ExternalOutput":
            shape = tuple(alloc.tensor_shape)
            dtype = mybir.dt.np(alloc.dtype)
            out_names.append(name)
            out_avals.append(jax.core.ShapedArray(shape, dtype))
            zero_outs.append(np.zeros(shape, dtype))
    n_params = len(in_names)
    all_names = in_names + out_names
    if partition_name is not None:
        all_names = all_names + [partition_name]

    def _body(*args):
        operands = list(args)
        if partition_name is not None:
            operands.append(bass2jax.partition_id_tensor())
        outs = bass2jax._bass_exec_p.bind(
            *operands,
            out_avals=tuple(out_avals),
            in_names=tuple(all_names),
            out_names=tuple(out_names),
            lowering_input_output_aliases=(),
            sim_require_finite=True,
            sim_require_nnan=True,
            nc=nc,
        )
        return tuple(outs)

    devices = jax.devices()[:N_CORES]
    mesh = Mesh(np.asarray(devices), ("core",))
    fn = jax.jit(
        shard_map(
            _body,
            mesh=mesh,
            in_specs=(PartitionSpec("core"),) * (n_params + len(out_names)),
            out_specs=(PartitionSpec("core"),) * len(out_names),
            check_rep=False,
        ),
        keep_unused=True,
    )
    runner = {
        "fn": fn,
        "in_names": in_names,
        "out_names": out_names,
        "out_avals": out_avals,
        "zero_outs": zero_outs,
        "jax": jax,
    }
    _RUNNER[t] = runner
    return runner


def run_device(in_maps, t=T):
    r = _get_runner(t)
    concat_in = [
        np.concatenate([np.asarray(m[name]) for m in in_maps], axis=0)
        for name in r["in_names"]
    ]
    concat_zero = [
        np.zeros((N_CORES * z.shape[0], *z.shape[1:]), z.dtype) for z in r["zero_outs"]
    ]
    outs = r["fn"](*concat_in, *concat_zero)
    results = []
    for c in range(N_CORES):
        results.append(
            {
                name: np.asarray(outs[i]).reshape(N_CORES, *r["out_avals"][i].shape)[c]
                for i, name in enumerate(r["out_names"])
            }
        )
    return results


def kernel(x, Wq, Wk, Wv, Wo, qn_w, kn_w):
    x = np.asarray(x, np.float32)
    in_maps = make_in_maps(
        x,
        np.asarray(Wq, np.float32),
        np.asarray(Wk, np.float32),
        np.asarray(Wv, np.float32),
        np.asarray(Wo, np.float32),
        np.asarray(qn_w, np.float32),
        np.asarray(kn_w, np.float32),
    )
    results = run_device(in_maps)
    return assemble(results)


# revision 7
# speedup vs baseline: 174.7114x; 174.7114x over previous
"""Trainium2 Bass kernel for causal self-attention (RoPE + per-head RMSNorm).

Reference computation (B=2, T=2048, C=1024, H=16, D=64):
    q = rope(rmsnorm(x @ Wq.T)); k = rope(rmsnorm(x @ Wk.T)); v = x @ Wv.T
    out = softmax(causal(q k^T / sqrt(D))) v @ Wo.T

Sharding over 8 NeuronCores: core c -> batch b = c//4, head-group g = c%4
(4 heads = 256 features per group).  Everything on-chip is computed in a
feature-major ("transposed") layout so no PE transposes are needed:
  - scores are computed as S^T[tk, tq] tiles, softmax runs over the
    partition axis using matmul-with-ones tricks (denominator comes from a
    ones column appended to V), and the final division is applied via a
    K=1 broadcast matmul.
  - attention output Y^T (feature-major) is exchanged with an AllToAll
    within each batch's 4-core group, giving each core the full 1024
    features for its 512-token slice; o_proj is computed on that slice.
Host side: shards/transposes inputs (bf16), assembles the fp32 output.
"""

import os
import sys

for _p in ("/opt/trn_rl_repo", "/root/.axon_site/_ro/trn_rl_repo"):
    if os.path.isdir(_p) and _p not in sys.path:
        sys.path.insert(0, _p)

import numpy as np
import ml_dtypes

import concourse.bass as bass
from concourse import bacc
import concourse.tile as tile
import concourse.mybir as mybir

BF16 = mybir.dt.bfloat16
F32 = mybir.dt.float32
AF = mybir.ActivationFunctionType

B, T, C, H, D = 2, 2048, 1024, 16, 64
N_CORES = 8
GH = 4  # heads per core
GF = GH * D  # features per core (256)
TB = 512  # token block (matmul N)
KT = C // 128  # 8 contraction k-tiles
EPS = float(np.finfo(np.float32).eps)
ROPE_BASE = 10000.0


def build_nc(t=T):
    ntb = t // TB  # tq blocks
    ntt = t // 128  # token 128-tiles
    tsl = t // 4  # per-core token slice for o_proj

    nc = bacc.Bacc("TRN2", target_bir_lowering=False, debug=False, num_devices=N_CORES)

    xt = nc.dram_tensor("xt", [C, t], BF16, kind="ExternalInput")
    wq = nc.dram_tensor("wq", [C, GF], BF16, kind="ExternalInput")
    wk = nc.dram_tensor("wk", [C, GF], BF16, kind="ExternalInput")
    wv = nc.dram_tensor("wv", [C, GF], BF16, kind="ExternalInput")
    wo = nc.dram_tensor("wo", [2 * C, C], BF16, kind="ExternalInput")
    cosf = nc.dram_tensor("cosf", [128, t], BF16, kind="ExternalInput")
    sinf = nc.dram_tensor("sinf", [128, t], BF16, kind="ExternalInput")
    pswap = nc.dram_tensor("pswap", [128, 128], BF16, kind="ExternalInput")
    blk2 = nc.dram_tensor("blk2", [128, 2], BF16, kind="ExternalInput")
    eqb = nc.dram_tensor("eqb", [2, 128], BF16, kind="ExternalInput")
    ekb = nc.dram_tensor("ekb", [2, 128], BF16, kind="ExternalInput")
    maskt = nc.dram_tensor("maskt", [128, 4 * TB], BF16, kind="ExternalInput")
    out = nc.dram_tensor("out", [C, tsl], F32, kind="ExternalOutput")

    with tile.TileContext(nc) as tc:
        with (
            nc.allow_low_precision(reason="bf16 compute by design"),
            tc.tile_pool(name="p_xt", bufs=KT) as p_xt,
            tc.tile_pool(name="p_w", bufs=KT) as p_w,
            tc.tile_pool(name="p_wo", bufs=2 * KT) as p_wo,
            tc.tile_pool(name="p_tab", bufs=1) as p_tab,
            tc.tile_pool(name="p_qk", bufs=2) as p_qk,
            tc.tile_pool(name="p_v", bufs=ntt) as p_v,
            tc.tile_pool(name="p_y", bufs=2) as p_y,
            tc.tile_pool(name="p_yg", bufs=2 * KT) as p_yg,
            tc.tile_pool(name="p_pt", bufs=4) as p_pt,
            tc.tile_pool(name="p_tmp", bufs=3) as p_tmp,
            tc.tile_pool(name="p_mm", bufs=6, space="PSUM") as p_mm,
            tc.tile_pool(name="p_oacc", bufs=2, space="PSUM") as p_oacc,
            tc.tile_pool(name="p_dram", bufs=2, space="DRAM") as p_dram,
        ):
            # ---- load inputs -------------------------------------------------
            xt_sb = []
            wq_sb = []
            wk_sb = []
            wv_sb = []
            for ct in range(KT):
                w_t = p_w.tile([128, GF], BF16, tag="wq")
                nc.sync.dma_start(w_t[:], wq[ct * 128 : (ct + 1) * 128, :])
                wq_sb.append(w_t)
                x_t = p_xt.tile([128, t], BF16, tag="xt")
                nc.sync.dma_start(x_t[:], xt[ct * 128 : (ct + 1) * 128, :])
                xt_sb.append(x_t)
            for ct in range(KT):
                w_t = p_w.tile([128, GF], BF16, tag="wk")
                nc.sync.dma_start(w_t[:], wk[ct * 128 : (ct + 1) * 128, :])
                wk_sb.append(w_t)
            for ct in range(KT):
                w_t = p_w.tile([128, GF], BF16, tag="wv")
                nc.sync.dma_start(w_t[:], wv[ct * 128 : (ct + 1) * 128, :])
                wv_sb.append(w_t)
            wo_sb = []
            for mt in range(2 * KT):
                w_t = p_wo.tile([128, C], BF16, tag="wo")
                nc.sync.dma_start(w_t[:], wo[mt * 128 : (mt + 1) * 128, :])
                wo_sb.append(w_t)

            cos_sb = p_tab.tile([128, t], BF16, tag="cos")
            nc.sync.dma_start(cos_sb[:], cosf[:])
            sin_sb = p_tab.tile([128, t], BF16, tag="sin")
            nc.sync.dma_start(sin_sb[:], sinf[:])
            pswap_sb = p_tab.tile([128, 128], BF16, tag="pswap")
            nc.sync.dma_start(pswap_sb[:], pswap[:])
            blk2_sb = p_tab.tile([128, 2], BF16, tag="blk2")
            nc.sync.dma_start(blk2_sb[:], blk2[:])
            eqb_sb = p_tab.tile([2, 128], BF16, tag="eqb")
            nc.sync.dma_start(eqb_sb[:], eqb[:])
            ekb_sb = p_tab.tile([2, 128], BF16, tag="ekb")
            nc.sync.dma_start(ekb_sb[:], ekb[:])
            mask_sb = p_tab.tile([128, 4 * TB], BF16, tag="mask")
            nc.sync.dma_start(mask_sb[:], maskt[:])
            ones64 = p_tab.tile([1, 64], BF16, tag="ones64")
            nc.vector.memset(ones64[:], 1.0)
            eps_sb = p_tab.tile([128, 1], F32, tag="eps")
            nc.vector.memset(eps_sb[:], EPS)

            # ---- q/k projections + rmsnorm + rope (feature-major) -----------
            def qk_tensor(w_sb, eb_sb):
                out_tiles = []
                for mt in range(2):  # 128-feature row groups (2 heads each)
                    qh_t = p_qk.tile([128, t], BF16, tag=f"qk{len(out_tiles)}")
                    for j in range(ntb):
                        jb = slice(j * TB, (j + 1) * TB)
                        pq = p_mm.tile([128, TB], F32, tag="mm")
                        for ct in range(KT):
                            nc.tensor.matmul(
                                pq[:],
                                w_sb[ct][:, mt * 128 : (mt + 1) * 128],
                                xt_sb[ct][:, jb],
                                start=(ct == 0),
                                stop=(ct == KT - 1),
                            )
                        # sum of squares per head (via blockdiag-ones matmul)
                        sq = p_tmp.tile([128, TB], BF16, tag="sq")
                        nc.scalar.activation(sq[:], pq[:], AF.Square)
                        pss = p_mm.tile([128, TB], F32, tag="mm")
                        nc.tensor.matmul(
                            pss[0:2, :], blk2_sb[:], sq[:], start=True, stop=True
                        )
                        sqm = p_tmp.tile([2, TB], F32, tag="sqm")
                        nc.scalar.activation(
                            sqm[:], pss[0:2, :], AF.Sqrt, scale=1.0 / D,
                            bias=eps_sb[0:2, :],
                        )
                        inv = p_tmp.tile([2, TB], BF16, tag="inv")
                        nc.vector.reciprocal(inv[:], sqm[:])
                        # broadcast inv over the 64 rows of each head (* norm w)
                        pinvb = p_mm.tile([128, TB], F32, tag="mm")
                        nc.tensor.matmul(
                            pinvb[:], eb_sb[:], inv[:], start=True, stop=True
                        )
                        invb = p_tmp.tile([128, TB], BF16, tag="invb")
                        nc.vector.tensor_copy(invb[:], pinvb[:])
                        qn = p_tmp.tile([128, TB], BF16, tag="qn")
                        nc.vector.tensor_mul(qn[:], pq[:], invb[:])
                        # rope: qh = qn*cos + swap(qn)*sin_signed
                        pqs = p_mm.tile([128, TB], F32, tag="mm")
                        nc.tensor.matmul(pqs[:], pswap_sb[:], qn[:], start=True, stop=True)
                        t1 = p_tmp.tile([128, TB], BF16, tag="t1")
                        nc.vector.tensor_mul(t1[:], qn[:], cos_sb[:, jb])
                        t2 = p_tmp.tile([128, TB], BF16, tag="t2")
                        nc.vector.tensor_mul(t2[:], pqs[:], sin_sb[:, jb])
                        nc.vector.tensor_add(qh_t[:, jb], t1[:], t2[:])
                    out_tiles.append(qh_t)
                return out_tiles

            qh_sb = qk_tensor(wq_sb, eqb_sb)
            kh_sb = qk_tensor(wk_sb, ekb_sb)

            # ---- v projection (token-major, [tk, 4x(64+ones)] layout) -------
            v_sb = []
            for tt in range(ntt):
                pv = p_mm.tile([128, TB], F32, tag="mm")
                for ct in range(KT):
                    nc.tensor.matmul(
                        pv[:, 0:GF],
                        xt_sb[ct][:, tt * 128 : (tt + 1) * 128],
                        wv_sb[ct][:],
                        start=(ct == 0),
                        stop=(ct == KT - 1),
                    )
                v_t = p_v.tile([128, GH * (D + 1)], BF16, tag="v")
                src = pv[:, 0:GF].rearrange("p (h d) -> p h d", h=GH)
                dst = v_t[:].rearrange("p (h d) -> p h d", h=GH, d=D + 1)
                nc.vector.tensor_copy(dst[:, :, 0:D], src)
                nc.vector.memset(dst[:, :, D : D + 1], 1.0)
                v_sb.append(v_t)

            # ---- attention + A2A exchange + o_proj --------------------------
            yg_sb = [None] * (2 * KT)
            bounce_in = []
            bounce_out = []
            for hp in range(2):  # head pairs (2 heads each)
                y_t = p_y.tile([128, t], BF16, tag="y")
                for j in range(ntb):
                    jb = slice(j * TB, (j + 1) * TB)
                    po = [
                        p_oacc.tile([D + 1, TB], F32, tag="oacc", name=f"po{i}")
                        for i in range(2)
                    ]
                    n_tt = 4 * (j + 1)
                    for tt in range(n_tt):
                        ps = [
                            p_mm.tile([128, TB], F32, tag="mm", name=f"ps{i}")
                            for i in range(2)
                        ]
                        for hl in range(2):  # head-in-pair
                            hofs = hl * 64
                            nc.tensor.matmul(
                                ps[hl][:],
                                kh_sb[hp][hofs : hofs + 64, tt * 128 : (tt + 1) * 128],
                                qh_sb[hp][hofs : hofs + 64, jb],
                                start=True,
                                stop=True,
                                tile_position=(hofs, 0),
                            )
                        for hl in range(2):
                            pt = p_pt.tile([128, TB], BF16, tag="pt")
                            nc.scalar.activation(
                                pt[:], ps[hl][:], AF.Exp, scale=1.0 / np.sqrt(D)
                            )
                            r = tt - 4 * j
                            if r >= 0:  # diagonal tile: apply causal mask
                                nc.vector.tensor_mul(
                                    pt[:], pt[:], mask_sb[:, r * TB : (r + 1) * TB]
                                )
                            h = 2 * hp + hl
                            nc.tensor.matmul(
                                po[hl][:],
                                v_sb[tt][:, h * (D + 1) : (h + 1) * (D + 1)],
                                pt[:],
                                start=(tt == 0),
                                stop=(tt == n_tt - 1),
                            )
                    # normalize: Y = O / den (den is row 64 of each po)
                    rec = [
                        p_tmp.tile([1, TB], BF16, tag=f"rec{i}", name=f"rec{i}")
                        for i in range(2)
                    ]
                    for hl in range(2):
                        nc.vector.reciprocal(rec[hl][:], po[hl][64:65, :])
                    pr = p_mm.tile([128, TB], F32, tag="mm")
                    nc.tensor.matmul(
                        pr[0:64, :], ones64[:], rec[0][:], start=True, stop=True,
                        tile_position=(0, 0),
                    )
                    nc.tensor.matmul(
                        pr[64:128, :], ones64[:], rec[1][:], start=True, stop=True,
                        tile_position=(0, 64),
                    )
                    r_sb = p_tmp.tile([128, TB], BF16, tag="rsb")
                    nc.vector.tensor_copy(r_sb[:], pr[:])
                    nc.vector.tensor_mul(y_t[0:64, jb], po[0][0:64, :], r_sb[0:64, :])
                    nc.vector.tensor_mul(
                        y_t[64:128, jb], po[1][0:64, :], r_sb[64:128, :]
                    )

                # exchange: 8-way AllToAll (4-core groups unsupported).
                # Shard s (dest rank s) carries our features(hp) for token
                # block s%4; rank c thus receives its token block c%4 from
                # every rank.  Rows from other-batch ranks are junk -- the
                # host zeroes the matching rows of wo so o_proj ignores them.
                bin_t = p_dram.tile([8 * 128, tsl], BF16, tag=f"bin{hp}")
                bout_t = p_dram.tile([8 * 128, tsl], BF16, tag=f"bout{hp}")
                bounce_in.append(bin_t)
                bounce_out.append(bout_t)
                for s in range(8):
                    i = s % 4
                    nc.gpsimd.dma_start(
                        bin_t[s * 128 : (s + 1) * 128, :],
                        y_t[:, i * tsl : (i + 1) * tsl],
                    )
                nc.gpsimd.collective_compute(
                    "AllToAll",
                    mybir.AluOpType.bypass,
                    ins=[bin_t.opt()],
                    outs=[bout_t.opt()],
                    replica_groups=[[0, 1, 2, 3, 4, 5, 6, 7]],
                )
                for i in range(8):
                    yg_t = p_yg.tile([128, tsl], BF16, tag="yg")
                    nc.sync.dma_start(yg_t[:], bout_t[i * 128 : (i + 1) * 128, :])
                    yg_sb[2 * i + hp] = yg_t

            # o_proj over our token slice: out^T[cout, tsl]
            for co in range(KT):
                pout = p_mm.tile([128, tsl], F32, tag="mm")
                for mt in range(2 * KT):
                    nc.tensor.matmul(
                        pout[:, 0:tsl],
                        wo_sb[mt][:, co * 128 : (co + 1) * 128],
                        yg_sb[mt][:],
                        start=(mt == 0),
                        stop=(mt == 2 * KT - 1),
                    )
                o_sb = p_tmp.tile([128, tsl], F32, tag="osb")
                nc.vector.tensor_copy(o_sb[:], pout[:, 0:tsl])
                nc.sync.dma_start(out[co * 128 : (co + 1) * 128, :], o_sb[:])

    nc.compile()
    return nc


# ---------------------------------------------------------------------------
# host side
# ---------------------------------------------------------------------------


def _rope_tables(t):
    inv_freq = 1.0 / (ROPE_BASE ** (np.arange(0, D, 2, dtype=np.float64) / D))  # [32]
    ang = np.arange(t, dtype=np.float64)[:, None] * inv_freq[None, :]  # [t, 32]
    cos = np.cos(ang).astype(np.float32)
    sin = np.sin(ang).astype(np.float32)
    cosf = np.empty((128, t), np.float32)
    sinf = np.empty((128, t), np.float32)
    for r in range(128):
        d = r % 64
        f = d if d < 32 else d - 32
        cosf[r] = cos[:, f]
        sinf[r] = -sin[:, f] if d < 32 else sin[:, f]
    return cosf, sinf


def _consts(t):
    cosf, sinf = _rope_tables(t)
    pswap = np.zeros((128, 128), np.float32)
    for j in range(128):
        d = j % 64
        i = (j - 32) if d >= 32 else (j + 32)
        pswap[i, j] = 1.0
    blk2 = np.zeros((128, 2), np.float32)
    blk2[0:64, 0] = 1.0
    blk2[64:128, 1] = 1.0
    maskt = np.zeros((128, 4 * TB), np.float32)
    for r in range(4):
        for p in range(128):
            lo = 128 * r + p
            if lo < TB:
                maskt[p, r * TB + lo : (r + 1) * TB] = 1.0
    return cosf, sinf, pswap, blk2, maskt


def _eb(w):
    e = np.zeros((2, 128), np.float32)
    e[0, 0:64] = w[0:64]
    e[1, 64:128] = w[0:64] if len(w) == 64 else w[64:128]
    return e


def _bf(x):
    return np.ascontiguousarray(x).astype(ml_dtypes.bfloat16)


def make_in_maps(x, Wq, Wk, Wv, Wo, qn_w, kn_w, t=T):
    cosf, sinf, pswap, blk2, maskt = _consts(t)
    eq = _eb(qn_w)
    ek = _eb(kn_w)
    common = {
        "cosf": _bf(cosf),
        "sinf": _bf(sinf),
        "pswap": _bf(pswap),
        "blk2": _bf(blk2),
        "eqb": _bf(eq),
        "ekb": _bf(ek),
        "maskt": _bf(maskt),
    }
    in_maps = []
    for c in range(N_CORES):
        b, g = c // 4, c % 4
        fs = slice(GF * g, GF * (g + 1))
        wot = Wo.T  # [c_in, c_out]
        wo_core = np.zeros((2 * C, C), np.float32)
        for i in range(8):
            if i // 4 == b:
                gi = i % 4
                for hp in range(2):
                    u = 256 * i + 128 * hp
                    f0 = GF * gi + 128 * hp
                    wo_core[u : u + 128, :] = wot[f0 : f0 + 128, :]
        in_maps.append(
            dict(
                common,
                xt=_bf(x[b, :t, :].T),
                wq=_bf(Wq[fs, :].T),
                wk=_bf(Wk[fs, :].T),
                wv=_bf(Wv[fs, :].T),
                wo=_bf(wo_core),
            )
        )
    return in_maps


def assemble(results, t=T):
    tsl = t // 4
    out = np.empty((B, t, C), np.float32)
    for c in range(N_CORES):
        b, g = c // 4, c % 4
        out[b, g * tsl : (g + 1) * tsl, :] = results[c]["out"].T
    return out


# -- cached PJRT runner (compile once, reuse across kernel() calls) ---------

_RUNNER = {}


def _get_runner(t=T):
    if t in _RUNNER:
        return _RUNNER[t]
    import jax
    from jax.sharding import Mesh, PartitionSpec
    from jax.experimental.shard_map import shard_map
    from concourse import bass2jax

    nc = build_nc(t)
    bass2jax.install_neuronx_cc_hook()

    partition_name = nc.partition_id_tensor.name if nc.partition_id_tensor else None
    in_names = []
    out_names = []
    out_avals = []
    zero_outs = []
    for alloc in nc.m.functions[0].allocations:
        if not isinstance(alloc, mybir.MemoryLocationSet):
            continue
        name = alloc.memorylocations[0].name
        if alloc.kind == "ExternalInput":
            if name == partition_name:
                continue
            in_names.append(name)
        elif alloc.kind == "ExternalOutput":
            shape = tuple(alloc.tensor_shape)
            dtype = mybir.dt.np(alloc.dtype)
            out_names.append(name)
            out_avals.append(jax.core.ShapedArray(shape, dtype))
            zero_outs.append(np.zeros(shape, dtype))
    n_params = len(in_names)
    all_names = in_names + out_names
    if partition_name is not None:
        all_names = all_names + [partition_name]

    def _body(*args):
        operands = list(args)
        if partition_name is not None:
            operands.append(bass2jax.partition_id_tensor())
        outs = bass2jax._bass_exec_p.bind(
            *operands,
            out_avals=tuple(out_avals),
            in_names=tuple(all_names),
            out_names=tuple(out_names),
            lowering_input_output_aliases=(),
            sim_require_finite=True,
            sim_require_nnan=True,
            nc=nc,
        )
        return tuple(outs)

    devices = jax.devices()[:N_CORES]
    mesh = Mesh(np.asarray(devices), ("core",))
    fn = jax.jit(
        shard_map(
            _body,
            mesh=mesh,
            in_specs=(PartitionSpec("core"),) * (n_params + len(out_names)),
            out_specs=(PartitionSpec("core"),) * len(out_names),
            check_rep=False,
        ),
        keep_unused=True,
    )
    runner = {
        "fn": fn,
        "body": _body,
        "in_names": in_names,
        "out_names": out_names,
        "out_avals": out_avals,
        "zero_outs": zero_outs,
        "jax": jax,
    }
    _RUNNER[t] = runner
    return runner


def run_device(in_maps, t=T):
    r = _get_runner(t)
    concat_in = [
        np.concatenate([np.asarray(m[name]) for m in in_maps], axis=0)
        for name in r["in_names"]
    ]
    concat_zero = [
        np.zeros((N_CORES * z.shape[0], *z.shape[1:]), z.dtype) for z in r["zero_outs"]
    ]
    outs = r["fn"](*concat_in, *concat_zero)
    results = []
    for c in range(N_CORES):
        results.append(
            {
                name: np.asarray(outs[i]).reshape(N_CORES, *r["out_avals"][i].shape)[c]
                for i, name in enumerate(r["out_names"])
            }
        )
    return results


def kernel(x, Wq, Wk, Wv, Wo, qn_w, kn_w):
    x = np.asarray(x, np.float32)
    in_maps = make_in_maps(
        x,
        np.asarray(Wq, np.float32),
        np.asarray(Wk, np.float32),
        np.asarray(Wv, np.float32),
        np.asarray(Wo, np.float32),
        np.asarray(qn_w, np.float32),
        np.asarray(kn_w, np.float32),
    )
    results = run_device(in_maps)
    return assemble(results)


# revision 12
# speedup vs baseline: 202.3131x; 1.1580x over previous
"""Trainium2 Bass kernel for causal self-attention (RoPE + per-head RMSNorm).

Reference computation (B=2, T=2048, C=1024, H=16, D=64):
    q = rope(rmsnorm(x @ Wq.T)); k = rope(rmsnorm(x @ Wk.T)); v = x @ Wv.T
    out = softmax(causal(q k^T / sqrt(D))) v @ Wo.T

Sharding over 8 NeuronCores: core c -> batch b = c//4, head-group g = c%4
(4 heads = 256 features per group).  Everything on-chip is computed in a
feature-major ("transposed") layout so no PE transposes are needed:
  - scores are computed as S^T[tk, tq] tiles, softmax runs over the
    partition axis using matmul-with-ones tricks (denominator comes from a
    ones column appended to V), and the final division is applied via a
    K=1 broadcast matmul.
  - attention output Y^T (feature-major) is exchanged with an AllToAll
    within each batch's 4-core group, giving each core the full 1024
    features for its 512-token slice; o_proj is computed on that slice.
Host side: shards/transposes inputs (bf16), assembles the fp32 output.
"""

import os
import sys

for _p in ("/opt/trn_rl_repo", "/root/.axon_site/_ro/trn_rl_repo"):
    if os.path.isdir(_p) and _p not in sys.path:
        sys.path.insert(0, _p)

import numpy as np
import ml_dtypes

import concourse.bass as bass
from concourse import bacc
import concourse.tile as tile
import concourse.mybir as mybir

BF16 = mybir.dt.bfloat16
F32 = mybir.dt.float32
AF = mybir.ActivationFunctionType

B, T, C, H, D = 2, 2048, 1024, 16, 64
N_CORES = 8
GH = 4  # heads per core
GF = GH * D  # features per core (256)
TB = 512  # token block (matmul N)
KT = C // 128  # 8 contraction k-tiles
EPS = float(np.finfo(np.float32).eps)
ROPE_BASE = 10000.0


def build_nc(t=T):
    ntb = t // TB  # tq blocks
    ntt = t // 128  # token 128-tiles
    tsl = t // 4  # per-core token slice for o_proj

    nc = bacc.Bacc("TRN2", target_bir_lowering=False, debug=False, num_devices=N_CORES)

    xt = nc.dram_tensor("xt", [C, t], BF16, kind="ExternalInput")
    wq = nc.dram_tensor("wq", [C, GF], BF16, kind="ExternalInput")
    wk = nc.dram_tensor("wk", [C, GF], BF16, kind="ExternalInput")
    wv = nc.dram_tensor("wv", [C, GF], BF16, kind="ExternalInput")
    wo = nc.dram_tensor("wo", [2 * C, C], BF16, kind="ExternalInput")
    cosf = nc.dram_tensor("cosf", [128, t], BF16, kind="ExternalInput")
    sinf = nc.dram_tensor("sinf", [128, t], BF16, kind="ExternalInput")
    pswap = nc.dram_tensor("pswap", [128, 128], BF16, kind="ExternalInput")
    blk2 = nc.dram_tensor("blk2", [128, 2], BF16, kind="ExternalInput")
    eqb = nc.dram_tensor("eqb", [2, 128], BF16, kind="ExternalInput")
    ekb = nc.dram_tensor("ekb", [2, 128], BF16, kind="ExternalInput")
    maskt = nc.dram_tensor("maskt", [128, 4 * TB], BF16, kind="ExternalInput")
    out = nc.dram_tensor("out", [C, tsl], F32, kind="ExternalOutput")

    with tile.TileContext(nc) as tc:
        with (
            nc.allow_low_precision(reason="bf16 compute by design"),
            tc.tile_pool(name="p_xt", bufs=KT) as p_xt,
            tc.tile_pool(name="p_w", bufs=KT) as p_w,
            tc.tile_pool(name="p_wo", bufs=2 * KT) as p_wo,
            tc.tile_pool(name="p_tab", bufs=1) as p_tab,
            tc.tile_pool(name="p_qk", bufs=2) as p_qk,
            tc.tile_pool(name="p_v", bufs=ntt) as p_v,
            tc.tile_pool(name="p_y", bufs=2) as p_y,
            tc.tile_pool(name="p_yg", bufs=2 * KT) as p_yg,
            tc.tile_pool(name="p_pt", bufs=6) as p_pt,
            tc.tile_pool(name="p_tmp", bufs=2) as p_tmp,
            tc.tile_pool(name="p_mm", bufs=6, space="PSUM") as p_mm,
            tc.tile_pool(name="p_oacc", bufs=2, space="PSUM") as p_oacc,
            tc.tile_pool(name="p_dram", bufs=2, space="DRAM") as p_dram,
        ):
            # ---- load inputs -------------------------------------------------
            xt_sb = []
            wq_sb = []
            wk_sb = []
            wv_sb = []
            for ct in range(KT):
                w_t = p_w.tile([128, GF], BF16, tag="wq")
                nc.sync.dma_start(w_t[:], wq[ct * 128 : (ct + 1) * 128, :])
                wq_sb.append(w_t)
                x_t = p_xt.tile([128, t], BF16, tag="xt")
                nc.sync.dma_start(x_t[:], xt[ct * 128 : (ct + 1) * 128, :])
                xt_sb.append(x_t)
            for ct in range(KT):
                w_t = p_w.tile([128, GF], BF16, tag="wk")
                nc.sync.dma_start(w_t[:], wk[ct * 128 : (ct + 1) * 128, :])
                wk_sb.append(w_t)
            cos_sb = p_tab.tile([128, t], BF16, tag="cos")
            nc.sync.dma_start(cos_sb[:], cosf[:])
            sin_sb = p_tab.tile([128, t], BF16, tag="sin")
            nc.sync.dma_start(sin_sb[:], sinf[:])
            pswap_sb = p_tab.tile([128, 128], BF16, tag="pswap")
            nc.sync.dma_start(pswap_sb[:], pswap[:])
            blk2_sb = p_tab.tile([128, 2], BF16, tag="blk2")
            nc.sync.dma_start(blk2_sb[:], blk2[:])
            eqb_sb = p_tab.tile([2, 128], BF16, tag="eqb")
            nc.sync.dma_start(eqb_sb[:], eqb[:])
            ekb_sb = p_tab.tile([2, 128], BF16, tag="ekb")
            nc.sync.dma_start(ekb_sb[:], ekb[:])
            mask_sb = p_tab.tile([128, 4 * TB], BF16, tag="mask")
            nc.sync.dma_start(mask_sb[:], maskt[:])
            ones64 = p_tab.tile([1, 64], BF16, tag="ones64")
            nc.vector.memset(ones64[:], 1.0)
            eps_sb = p_tab.tile([128, 1], F32, tag="eps")
            nc.vector.memset(eps_sb[:], EPS)
            for ct in range(KT):
                w_t = p_w.tile([128, GF], BF16, tag="wv")
                nc.sync.dma_start(w_t[:], wv[ct * 128 : (ct + 1) * 128, :])
                wv_sb.append(w_t)
            wo_sb = []
            for mt in range(2 * KT):
                w_t = p_wo.tile([128, C], BF16, tag="wo")
                nc.sync.dma_start(w_t[:], wo[mt * 128 : (mt + 1) * 128, :])
                wo_sb.append(w_t)

            # ---- q/k/v projections, rmsnorm + rope (software-pipelined) -----
            # q/k blocks (proj matmuls) are emitted with their norm/rope
            # chain lagging one block, and one v-tile projection interleaved
            # per block, so PE always has independent matmuls to run while
            # ACT/DVE work through the chain.
            qh_sb = [p_qk.tile([128, t], BF16, tag="qk0", name="qh0"),
                     p_qk.tile([128, t], BF16, tag="qk1", name="qh1")]
            kh_sb = [p_qk.tile([128, t], BF16, tag="qk0", name="kh0"),
                     p_qk.tile([128, t], BF16, tag="qk1", name="kh1")]
            v_sb = []

            def emit_chain(pq, dst, jb, eb_sb):
                # sum of squares per head (via blockdiag-ones matmul)
                sq = p_tmp.tile([128, TB], BF16, tag="sq", name="sq")
                nc.scalar.activation(sq[:], pq[:], AF.Square)
                pss = p_mm.tile([128, TB], F32, tag="mm", name="pss")
                nc.tensor.matmul(pss[0:2, :], blk2_sb[:], sq[:], start=True, stop=True)
                sqm = p_tmp.tile([2, TB], F32, tag="sqm", name="sqm")
                nc.scalar.activation(
                    sqm[:], pss[0:2, :], AF.Sqrt, scale=1.0 / D, bias=eps_sb[0:2, :]
                )
                invf = p_tmp.tile([2, TB], F32, tag="invf", name="invf")
                nc.vector.reciprocal_approx_fast(out=invf[:], in_=sqm[:])
                inv = p_tmp.tile([2, TB], BF16, tag="inv", name="inv")
                nc.vector.tensor_copy(inv[:], invf[:])
                # broadcast inv over the 64 rows of each head (* norm w)
                pinvb = p_mm.tile([128, TB], F32, tag="mm", name="pinvb")
                nc.tensor.matmul(pinvb[:], eb_sb[:], inv[:], start=True, stop=True)
                invb = p_tmp.tile([128, TB], BF16, tag="invb", name="invb")
                nc.vector.tensor_copy(invb[:], pinvb[:])
                qn = p_tmp.tile([128, TB], BF16, tag="qn", name="qn")
                nc.vector.tensor_mul(qn[:], pq[:], invb[:])
                # rope: dst = qn*cos + swap(qn)*sin_signed
                pqs = p_mm.tile([128, TB], F32, tag="mm", name="pqs")
                nc.tensor.matmul(pqs[:], pswap_sb[:], qn[:], start=True, stop=True)
                t1 = p_tmp.tile([128, TB], BF16, tag="t1", name="t1")
                nc.vector.tensor_mul(t1[:], qn[:], cos_sb[:, jb])
                t2 = p_tmp.tile([128, TB], BF16, tag="t2", name="t2")
                nc.vector.tensor_mul(t2[:], pqs[:], sin_sb[:, jb])
                nc.vector.tensor_add(dst[:, jb], t1[:], t2[:])

            def emit_v(tt):
                pv = p_mm.tile([128, TB], F32, tag="mm", name="pv")
                for ct in range(KT):
                    nc.tensor.matmul(
                        pv[:, 0:GF],
                        xt_sb[ct][:, tt * 128 : (tt + 1) * 128],
                        wv_sb[ct][:],
                        start=(ct == 0),
                        stop=(ct == KT - 1),
                    )
                v_t = p_v.tile([128, GH * (D + 1)], BF16, tag="v", name="v_t")
                vsrc = pv[:, 0:GF].rearrange("p (h d) -> p h d", h=GH)
                vdst = v_t[:].rearrange("p (h d) -> p h d", h=GH, d=D + 1)
                nc.vector.tensor_copy(vdst[:, :, 0:D], vsrc)
                nc.vector.memset(vdst[:, :, D : D + 1], 1.0)
                v_sb.append(v_t)

            pending = None
            vb = 0
            for w_sb, eb_sb, dst_tiles in (
                (wq_sb, eqb_sb, qh_sb),
                (wk_sb, ekb_sb, kh_sb),
            ):
                for mt in range(2):
                    for j in range(ntb):
                        jb = slice(j * TB, (j + 1) * TB)
                        pq = p_mm.tile([128, TB], F32, tag="mm", name="pq")
                        for ct in range(KT):
                            nc.tensor.matmul(
                                pq[:],
                                w_sb[ct][:, mt * 128 : (mt + 1) * 128],
                                xt_sb[ct][:, jb],
                                start=(ct == 0),
                                stop=(ct == KT - 1),
                            )
                        if pending is not None:
                            emit_chain(*pending)
                        pending = (pq, dst_tiles[mt], jb, eb_sb)
                        if vb < ntt:
                            emit_v(vb)
                            vb += 1
            emit_chain(*pending)
            while vb < ntt:
                emit_v(vb)
                vb += 1

            # ---- attention + A2A exchange + o_proj --------------------------
            yg_sb = [None] * (2 * KT)
            bounce_in = []
            bounce_out = []
            for hp in range(2):  # head pairs (2 heads each)
                y_t = p_y.tile([128, t], BF16, tag="y")
                for j in range(ntb):
                    jb = slice(j * TB, (j + 1) * TB)
                    po = [
                        p_oacc.tile([D + 1, TB], F32, tag="oacc", name=f"po{i}")
                        for i in range(2)
                    ]
                    n_tt = 4 * (j + 1)

                    def attn_v(pts, tt):
                        for hl in range(2):
                            h = 2 * hp + hl
                            nc.tensor.matmul(
                                po[hl][:],
                                v_sb[tt][:, h * (D + 1) : (h + 1) * (D + 1)],
                                pts[hl][:],
                                start=(tt == 0),
                                stop=(tt == n_tt - 1),
                            )

                    pend = None
                    for tt in range(n_tt):
                        ps = [
                            p_mm.tile([128, TB], F32, tag="mm", name=f"ps{i}")
                            for i in range(2)
                        ]
                        for hl in range(2):  # head-in-pair
                            hofs = hl * 64
                            nc.tensor.matmul(
                                ps[hl][:],
                                kh_sb[hp][hofs : hofs + 64, tt * 128 : (tt + 1) * 128],
                                qh_sb[hp][hofs : hofs + 64, jb],
                                start=True,
                                stop=True,
                                tile_position=(hofs, 0),
                            )
                        pts = []
                        for hl in range(2):
                            pt = p_pt.tile([128, TB], BF16, tag="pt")
                            nc.scalar.activation(
                                pt[:], ps[hl][:], AF.Exp, scale=1.0 / np.sqrt(D)
                            )
                            r = tt - 4 * j
                            if r >= 0:  # diagonal tile: apply causal mask
                                nc.vector.tensor_mul(
                                    pt[:], pt[:], mask_sb[:, r * TB : (r + 1) * TB]
                                )
                            pts.append(pt)
                        if pend is not None:
                            attn_v(*pend)
                        pend = (pts, tt)
                    attn_v(*pend)
                    # normalize: Y = O / den (den is row 64 of each po)
                    rec = []
                    for hl in range(2):
                        dn = p_tmp.tile([1, TB], F32, tag=f"den{hl}", name=f"dn{hl}")
                        nc.vector.tensor_copy(dn[:], po[hl][64:65, :])
                        rf = p_tmp.tile([1, TB], F32, tag=f"recf{hl}", name=f"rf{hl}")
                        nc.vector.reciprocal_approx_fast(out=rf[:], in_=dn[:])
                        rc = p_tmp.tile([1, TB], BF16, tag=f"rec{hl}", name=f"rc{hl}")
                        nc.vector.tensor_copy(rc[:], rf[:])
                        rec.append(rc)
                    pr = p_mm.tile([128, TB], F32, tag="mm")
                    nc.tensor.matmul(
                        pr[0:64, :], ones64[:], rec[0][:], start=True, stop=True,
                        tile_position=(0, 0),
                    )
                    nc.tensor.matmul(
                        pr[64:128, :], ones64[:], rec[1][:], start=True, stop=True,
                        tile_position=(0, 64),
                    )
                    r_sb = p_tmp.tile([128, TB], BF16, tag="rsb")
                    nc.vector.tensor_copy(r_sb[:], pr[:])
                    nc.vector.tensor_mul(y_t[0:64, jb], po[0][0:64, :], r_sb[0:64, :])
                    nc.vector.tensor_mul(
                        y_t[64:128, jb], po[1][0:64, :], r_sb[64:128, :]
                    )

                # exchange: 8-way AllToAll (4-core groups unsupported).
                # Shard s (dest rank s) carries our features(hp) for token
                # block s%4; rank c thus receives its token block c%4 from
                # every rank.  Rows from other-batch ranks are junk -- the
                # host zeroes the matching rows of wo so o_proj ignores them.
                bin_t = p_dram.tile([8 * 128, tsl], BF16, tag=f"bin{hp}")
                bout_t = p_dram.tile([8 * 128, tsl], BF16, tag=f"bout{hp}")
                bounce_in.append(bin_t)
                bounce_out.append(bout_t)
                for s in range(8):
                    i = s % 4
                    nc.gpsimd.dma_start(
                        bin_t[s * 128 : (s + 1) * 128, :],
                        y_t[:, i * tsl : (i + 1) * tsl],
                    )
                nc.gpsimd.collective_compute(
                    "AllToAll",
                    mybir.AluOpType.bypass,
                    ins=[bin_t.opt()],
                    outs=[bout_t.opt()],
                    replica_groups=[[0, 1, 2, 3, 4, 5, 6, 7]],
                )
                for i in range(8):
                    yg_t = p_yg.tile([128, tsl], BF16, tag="yg")
                    nc.sync.dma_start(yg_t[:], bout_t[i * 128 : (i + 1) * 128, :])
                    yg_sb[2 * i + hp] = yg_t

            # o_proj over our token slice: out^T[cout, tsl]
            mt_order = [2 * i for i in range(KT)] + [2 * i + 1 for i in range(KT)]
            for co in range(KT):
                pout = p_mm.tile([128, tsl], F32, tag="mm")
                for n_mt, mt in enumerate(mt_order):
                    nc.tensor.matmul(
                        pout[:, 0:tsl],
                        wo_sb[mt][:, co * 128 : (co + 1) * 128],
                        yg_sb[mt][:],
                        start=(n_mt == 0),
                        stop=(n_mt == 2 * KT - 1),
                    )
                o_sb = p_tmp.tile([128, tsl], F32, tag="osb")
                nc.vector.tensor_copy(o_sb[:], pout[:, 0:tsl])
                nc.sync.dma_start(out[co * 128 : (co + 1) * 128, :], o_sb[:])

    nc.compile()
    return nc


# ---------------------------------------------------------------------------
# host side
# ---------------------------------------------------------------------------


def _rope_tables(t):
    inv_freq = 1.0 / (ROPE_BASE ** (np.arange(0, D, 2, dtype=np.float64) / D))  # [32]
    ang = np.arange(t, dtype=np.float64)[:, None] * inv_freq[None, :]  # [t, 32]
    cos = np.cos(ang).astype(np.float32)
    sin = np.sin(ang).astype(np.float32)
    cosf = np.empty((128, t), np.float32)
    sinf = np.empty((128, t), np.float32)
    for r in range(128):
        d = r % 64
        f = d if d < 32 else d - 32
        cosf[r] = cos[:, f]
        sinf[r] = -sin[:, f] if d < 32 else sin[:, f]
    return cosf, sinf


def _consts(t):
    cosf, sinf = _rope_tables(t)
    pswap = np.zeros((128, 128), np.float32)
    for j in range(128):
        d = j % 64
        i = (j - 32) if d >= 32 else (j + 32)
        pswap[i, j] = 1.0
    blk2 = np.zeros((128, 2), np.float32)
    blk2[0:64, 0] = 1.0
    blk2[64:128, 1] = 1.0
    maskt = np.zeros((128, 4 * TB), np.float32)
    for r in range(4):
        for p in range(128):
            lo = 128 * r + p
            if lo < TB:
                maskt[p, r * TB + lo : (r + 1) * TB] = 1.0
    return cosf, sinf, pswap, blk2, maskt


def _eb(w):
    e = np.zeros((2, 128), np.float32)
    e[0, 0:64] = w[0:64]
    e[1, 64:128] = w[0:64] if len(w) == 64 else w[64:128]
    return e


def _bf(x):
    return np.ascontiguousarray(x).astype(ml_dtypes.bfloat16)


def make_in_maps(x, Wq, Wk, Wv, Wo, qn_w, kn_w, t=T):
    cosf, sinf, pswap, blk2, maskt = _consts(t)
    eq = _eb(qn_w)
    ek = _eb(kn_w)
    common = {
        "cosf": _bf(cosf),
        "sinf": _bf(sinf),
        "pswap": _bf(pswap),
        "blk2": _bf(blk2),
        "eqb": _bf(eq),
        "ekb": _bf(ek),
        "maskt": _bf(maskt),
    }
    in_maps = []
    for c in range(N_CORES):
        b, g = c // 4, c % 4
        fs = slice(GF * g, GF * (g + 1))
        wot = Wo.T  # [c_in, c_out]
        wo_core = np.zeros((2 * C, C), np.float32)
        for i in range(8):
            if i // 4 == b:
                gi = i % 4
                for hp in range(2):
                    u = 256 * i + 128 * hp
                    f0 = GF * gi + 128 * hp
                    wo_core[u : u + 128, :] = wot[f0 : f0 + 128, :]
        in_maps.append(
            dict(
                common,
                xt=_bf(x[b, :t, :].T),
                wq=_bf(Wq[fs, :].T),
                wk=_bf(Wk[fs, :].T),
                wv=_bf(Wv[fs, :].T),
                wo=_bf(wo_core),
            )
        )
    return in_maps


def assemble(results, t=T):
    tsl = t // 4
    out = np.empty((B, t, C), np.float32)
    for c in range(N_CORES):
        b, g = c // 4, c % 4
        out[b, g * tsl : (g + 1) * tsl, :] = results[c]["out"].T
    return out


# -- cached PJRT runner (compile once, reuse across kernel() calls) ---------

_RUNNER = {}


def _get_runner(t=T):
    if t in _RUNNER:
        return _RUNNER[t]
    import jax
    from jax.sharding import Mesh, PartitionSpec
    from jax.experimental.shard_map import shard_map
    from concourse import bass2jax

    nc = build_nc(t)
    bass2jax.install_neuronx_cc_hook()

    partition_name = nc.partition_id_tensor.name if nc.partition_id_tensor else None
    in_names = []
    out_names = []
    out_avals = []
    zero_outs = []
    for alloc in nc.m.functions[0].allocations:
        if not isinstance(alloc, mybir.MemoryLocationSet):
            continue
        name = alloc.memorylocations[0].name
        if alloc.kind == "ExternalInput":
            if name == partition_name:
                continue
            in_names.append(name)
        elif alloc.kind == "ExternalOutput":
            shape = tuple(alloc.tensor_shape)
            dtype = mybir.dt.np(alloc.dtype)
            out_names.append(name)
            out_avals.append(jax.core.ShapedArray(shape, dtype))
            zero_outs.append(np.zeros(shape, dtype))
    n_params = len(in_names)
    all_names = in_names + out_names
    if partition_name is not None:
        all_names = all_names + [partition_name]

    def _body(*args):
        operands = list(args)
        if partition_name is not None:
            operands.append(bass2jax.partition_id_tensor())
        outs = bass2jax._bass_exec_p.bind(
            *operands,
            out_avals=tuple(out_avals),
            in_names=tuple(all_names),
            out_names=tuple(out_names),
            lowering_input_output_aliases=(),
            sim_require_finite=True,
            sim_require_nnan=True,
            nc=nc,
        )
        return tuple(outs)

    devices = jax.devices()[:N_CORES]
    mesh = Mesh(np.asarray(devices), ("core",))
    fn = jax.jit(
        shard_map(
            _body,
            mesh=mesh,
            in_specs=(PartitionSpec("core"),) * (n_params + len(out_names)),
            out_specs=(PartitionSpec("core"),) * len(out_names),
            check_rep=False,
        ),
        keep_unused=True,
    )
    runner = {
        "fn": fn,
        "body": _body,
        "in_names": in_names,
        "out_names": out_names,
        "out_avals": out_avals,
        "zero_outs": zero_outs,
        "jax": jax,
    }
    _RUNNER[t] = runner
    return runner


def run_device(in_maps, t=T):
    r = _get_runner(t)
    concat_in = [
        np.concatenate([np.asarray(m[name]) for m in in_maps], axis=0)
        for name in r["in_names"]
    ]
    concat_zero = [
        np.zeros((N_CORES * z.shape[0], *z.shape[1:]), z.dtype) for z in r["zero_outs"]
    ]
    outs = r["fn"](*concat_in, *concat_zero)
    results = []
    for c in range(N_CORES):
        results.append(
            {
                name: np.asarray(outs[i]).reshape(N_CORES, *r["out_avals"][i].shape)[c]
                for i, name in enumerate(r["out_names"])
            }
        )
    return results


def kernel(x, Wq, Wk, Wv, Wo, qn_w, kn_w):
    x = np.asarray(x, np.float32)
    in_maps = make_in_maps(
        x,
        np.asarray(Wq, np.float32),
        np.asarray(Wk, np.float32),
        np.asarray(Wv, np.float32),
        np.asarray(Wo, np.float32),
        np.asarray(qn_w, np.float32),
        np.asarray(kn_w, np.float32),
    )
    results = run_device(in_maps)
    return assemble(results)
